# revision 11
# baseline (speedup 1.0000x reference)
"""Trainium2 Bass kernel for Exphormer sparse attention (GNN message passing).

Strategy (v3):
  - Nodes range-partitioned across 8 cores by dst; edges routed to the core
    owning their dst and grouped into 128-node dst blocks, so per-core
    segment sums are complete (no collective).
  - DRAM node table KVT[n, 264] bf16 per core: row = [K(128) | VI(136)],
    where VI interleaves V with ones (per head: 16 V dims then a 1.0), so a
    single per-edge multiply produces the fused [msg | Z] accumulation
    operand.  Biases ride a one-time broadcast prefill of KVT; the per-tile
    table write uses a CCE-add DMA, so no bias matmuls.
  - Per 128-edge tile: KV[src] rows via one indirect-DMA gather; Q[dst]
    expanded by a one-hot matmul.  One-hot matrices (both orientations) are
    precomputed on the host from the edge indices and DMA'd as fp8 -- no
    on-device one-hot builds.
  - T = K[src]*Q[dst] (DVE); transpose(T) (PE); T^T @ Wblk -> per-head
    [dw|db] into a grouped PSUM tile.  Score tail sc=clip(dw*a+db), exp
    batched over 6 tiles (4 instructions per group).
  - msg|Z = VI[src]*exp(sc) in one DVE op; segment-sum via one-hot matmul
    accumulated in PSUM per dst block.  Tiles per block are variable
    (max over cores per block index) to cut padding.
"""

import math
import os

import numpy as np
import ml_dtypes
import orjson

import concourse.bass as bass
import concourse.tile as tile
import concourse.mybir as mybir
from concourse.bass_utils import run_bass_kernel_spmd

_WAIT_LIMIT = 1
_fix_installed = False


def _split_waits(bir_json: bytes, limit: int = _WAIT_LIMIT) -> bytes:
    d = orjson.loads(bir_json)
    ctr = 0
    changed = False
    for fn in d.get("functions", []):
        for blk in fn.get("blocks", []):
            out = []
            for inst in blk.get("instructions", []):
                si = inst.get("sync_info")
                ow = (si or {}).get("on_wait") or []
                if si is not None and len(ow) > limit and "engine" in inst:
                    keep = ow[-limit:]
                    for w in ow[:-limit]:
                        ctr += 1
                        out.append({
                            "debug": inst.get("debug", 0),
                            "engine": inst["engine"],
                            "ins": [],
                            "outs": [],
                            "name": f"antsw-{ctr}-{inst['name']}",
                            "opcode": "EventSemaphore",
                            "sync_info": {"on_update": [], "on_wait": [w]},
                        })
                    si["on_wait"] = keep
                    changed = True
                out.append(inst)
            blk["instructions"] = out
    if not changed:
        return bir_json
    return orjson.dumps(d)


def _install_wait_fix():
    global _fix_installed
    if _fix_installed:
        return
    _fix_installed = True
    import concourse.bass_utils as bu
    import concourse.bass2jax as b2j

    orig = bu.compile_bir_kernel

    def wrapped(bir_json, tmpdir, neff_name="file.neff"):
        return orig(_split_waits(bytes(bir_json)), tmpdir, neff_name=neff_name)

    bu.compile_bir_kernel = wrapped
    b2j.compile_bir_kernel = wrapped


P = 128
ROW = 264          # table row: K(128) | VI(136)
F32 = mybir.dt.float32
BF16 = mybir.dt.bfloat16
FP8 = mybir.dt.float8e4
I32 = mybir.dt.int32
AX = mybir.AluOpType
AF = mybir.ActivationFunctionType

LAST_RESULTS = None  # test harness reads exec_time_ns from here


class Cfg:
    def __init__(self, n_nodes=50000, n_cores=8):
        self.n_nodes = n_nodes
        self.n_cores = n_cores
        self.npc = n_nodes // n_cores            # nodes per core
        self.nb = math.ceil(self.npc / P)        # dst blocks per core
        self.qrows = self.nb * P                 # padded local Q rows
        need = max(n_nodes, (n_cores - 1) * self.npc + self.qrows)
        self.npad = math.ceil(need / P) * P
        self.ntb = [1] * self.nb                 # tiles per block (host fills)
        self.base = [0] * self.nb                # tile base per block
        self.tt = self.nb                        # total edge tiles per core


def build_nc(cfg: Cfg):
    kvgbufs = int(os.environ.get("KERNEL_KVGBUFS", "8"))
    GB = int(os.environ.get("KERNEL_GB", "6"))   # tiles per score group
    nc = bass.Bass("TRN2", target_bir_lowering=False, num_devices=cfg.n_cores)

    XT = nc.dram_tensor("xt", [P, cfg.npad], BF16, kind="ExternalInput").ap()
    XTQ = nc.dram_tensor("xtq", [P, cfg.qrows], BF16, kind="ExternalInput").ap()
    WKV = nc.dram_tensor("wkv", [P, 256], BF16, kind="ExternalInput").ap()
    WQ = nc.dram_tensor("wq", [P, P], BF16, kind="ExternalInput").ap()
    BQR = nc.dram_tensor("bqr", [1, P], BF16, kind="ExternalInput").ap()
    ONESR = nc.dram_tensor("onesr", [1, P], BF16, kind="ExternalInput").ap()
    BKB = nc.dram_tensor("bkb", [P, P], F32, kind="ExternalInput").ap()
    BVB = nc.dram_tensor("bvb", [P, P], F32, kind="ExternalInput").ap()
    IDN = nc.dram_tensor("idn", [P, P], BF16, kind="ExternalInput").ap()
    WBLK = nc.dram_tensor("wblk", [P, 16], BF16, kind="ExternalInput").ap()
    SRC = nc.dram_tensor("srct", [P, cfg.tt], I32, kind="ExternalInput").ap()
    AT = nc.dram_tensor("at", [P, cfg.tt], F32, kind="ExternalInput").ap()
    OH2T = nc.dram_tensor("oh2t", [P, cfg.tt * P], FP8, kind="ExternalInput").ap()
    OHT = nc.dram_tensor("oht", [P, cfg.tt * P], FP8, kind="ExternalInput").ap()
    HOUT = nc.dram_tensor("hout", [cfg.qrows, P], F32, kind="ExternalOutput").ap()

    KVT = nc.dram_tensor("kvt", [cfg.npad, ROW], BF16).ap()

    n_kv_tiles = cfg.npad // P
    n_q_tiles = cfg.qrows // P
    SLAB = 8  # node tiles per x-slab load

    with tile.TileContext(nc) as tc:
        with (
            tc.tile_pool(name="const", bufs=1) as cpool,
            tc.tile_pool(name="meta", bufs=1) as mpool,
            tc.tile_pool(name="qres", bufs=1) as qpool,
        ):
            # ---- constants (all host-prepped) ----
            ident = cpool.tile([P, P], BF16)
            nc.sync.dma_start(out=ident[:], in_=IDN[:])
            ones_row = cpool.tile([1, P], BF16)
            nc.sync.dma_start(out=ones_row[:], in_=ONESR[:])
            wkv_sb = cpool.tile([P, 256], BF16)
            nc.sync.dma_start(out=wkv_sb[:], in_=WKV[:])
            wq_sb = cpool.tile([P, P], BF16)
            nc.sync.dma_start(out=wq_sb[:], in_=WQ[:])
            bq_sb = cpool.tile([1, P], BF16)
            nc.sync.dma_start(out=bq_sb[:], in_=BQR[:])
            wblk = cpool.tile([P, 16], BF16)
            nc.sync.dma_start(out=wblk[:], in_=WBLK[:])
            bkb = cpool.tile([P, P], F32)
            nc.sync.dma_start(out=bkb[:], in_=BKB[:])
            bvb = cpool.tile([P, P], F32)
            nc.sync.dma_start(out=bvb[:], in_=BVB[:])

            # ---- resident edge metadata ----
            src_sb = mpool.tile([P, cfg.tt], I32)
            nc.sync.dma_start(out=src_sb[:], in_=SRC[:])
            a_sb = mpool.tile([P, cfg.tt], F32)
            nc.sync.dma_start(out=a_sb[:], in_=AT[:])

            # ---- local Q, SBUF-resident [128, nb*128] ----
            qres = qpool.tile([P, cfg.qrows], BF16)

            with (
                tc.tile_pool(name="bld", bufs=3) as bpool,
                tc.tile_pool(name="bldp", bufs=2, space="PSUM") as bppool,
            ):
                for j in range(n_q_tiles):
                    if j % SLAB == 0:
                        xq = bpool.tile([P, SLAB * P], BF16, tag="xq")
                        w = min(SLAB * P, cfg.qrows - j * P)
                        nc.sync.dma_start(out=xq[:, :w], in_=XTQ[:, j * P:j * P + w])
                    lhs = xq[:, (j % SLAB) * P:(j % SLAB + 1) * P]
                    qp = bppool.tile([P, P], F32, tag="qp")
                    nc.tensor.matmul(out=qp[:], lhsT=lhs, rhs=wq_sb[:],
                                     start=True, stop=False)
                    nc.tensor.matmul(out=qp[:], lhsT=ones_row[:1, :], rhs=bq_sb[:1, :],
                                     start=False, stop=True)
                    if j % 2 == 0:
                        nc.scalar.activation(out=qres[:, j * P:(j + 1) * P],
                                             in_=qp[:], func=AF.Copy)
                    else:
                        nc.vector.tensor_copy(out=qres[:, j * P:(j + 1) * P],
                                              in_=qp[:])

                for i in range(n_kv_tiles):
                    if i % SLAB == 0:
                        xs = bpool.tile([P, SLAB * P], BF16, tag="xs")
                        w = min(SLAB * P, cfg.npad - i * P)
                        nc.sync.dma_start(out=xs[:, :w], in_=XT[:, i * P:i * P + w])
                    lhs = xs[:, (i % SLAB) * P:(i % SLAB + 1) * P]
                    bp = bppool.tile([P, 256], F32, tag="bp")
                    nc.tensor.matmul(out=bp[:], lhsT=lhs, rhs=wkv_sb[:],
                                     start=True, stop=True)
                    kvsb = bpool.tile([P, ROW], BF16, tag="kvsb")
                    vi = kvsb[:, 128:ROW].rearrange("p (h d) -> p h d", d=17)
                    if i < 3:  # set the ones-slot once per pool buffer
                        nc.gpsimd.memset(vi[:, :, 16:17], 1.0)
                    # K with bias (DVE add); V without bias (bias rides Z at
                    # readout: h = (wV' + bv*Z)/(Z+eps))
                    nc.vector.tensor_tensor(out=kvsb[:, 0:128], in0=bp[:, 0:128],
                                            in1=bkb[:], op=AX.add)
                    nc.scalar.activation(
                        out=vi[:, :, 0:16],
                        in_=bp[:, 128:256].rearrange("p (h d) -> p h d", d=16),
                        func=AF.Copy)
                    nc.sync.dma_start(out=KVT[i * P:(i + 1) * P, :], in_=kvsb[:])

            # ---- edge phase ----
            with (
                tc.tile_pool(name="kvg", bufs=kvgbufs) as kvpool,
                tc.tile_pool(name="work", bufs=4) as wpool,
                tc.tile_pool(name="ohb", bufs=2) as ohpool,
                tc.tile_pool(name="scb", bufs=2) as scbpool,
                tc.tile_pool(name="qxp", bufs=2, space="PSUM") as qxpool,
                tc.tile_pool(name="ttp", bufs=2, space="PSUM") as ttpool,
                tc.tile_pool(name="scp", bufs=2, space="PSUM") as scpool,
                tc.tile_pool(name="accp", bufs=2, space="PSUM") as accpool,
            ):
                ntmax = max(cfg.ntb)
                for blk in range(cfg.nb):
                    t0 = cfg.base[blk]
                    nt = cfg.ntb[blk]
                    qwin = qres[:, blk * P:(blk + 1) * P]
                    oh2b = ohpool.tile([P, ntmax * P], FP8, tag="oh2b")
                    nc.sync.dma_start(out=oh2b[:, 0:nt * P],
                                      in_=OH2T[:, t0 * P:(t0 + nt) * P])
                    ohb = ohpool.tile([P, ntmax * P], FP8, tag="ohb")
                    nc.sync.dma_start(out=ohb[:, 0:nt * P],
                                      in_=OHT[:, t0 * P:(t0 + nt) * P])
                    acc = accpool.tile([P, 136], F32, tag="acc")
                    n_grp = math.ceil(nt / GB)
                    for g in range(n_grp):
                        gt0 = g * GB
                        gn = min(GB, nt - gt0)
                        scps = scpool.tile([P, GB * 16], F32, tag="scps")
                        kvgs = []
                        for j in range(gn):
                            ti = gt0 + j
                            T = t0 + ti
                            kvg = kvpool.tile([P, ROW], BF16, tag="kvg")
                            kvgs.append(kvg)
                            nc.gpsimd.indirect_dma_start(
                                out=kvg[:], out_offset=None, in_=KVT[:],
                                in_offset=bass.IndirectOffsetOnAxis(
                                    ap=src_sb[:, T:T + 1], axis=0))
                            qx = qxpool.tile([P, P], F32, tag="qx")
                            nc.tensor.matmul(out=qx[:],
                                             lhsT=oh2b[:, ti * P:(ti + 1) * P],
                                             rhs=qwin, start=True, stop=True)
                            ttile = wpool.tile([P, P], BF16, tag="ttile")
                            nc.vector.tensor_tensor(
                                out=ttile[:], in0=kvg[:, 0:128],
                                in1=qx[:], op=AX.mult)
                            ttp = ttpool.tile([P, P], BF16, tag="ttp")
                            nc.tensor.transpose(out=ttp[:], in_=ttile[:],
                                                identity=ident[:])
                            tts = wpool.tile([P, P], BF16, tag="tts")
                            nc.scalar.activation(out=tts[:], in_=ttp[:], func=AF.Copy)
                            nc.tensor.matmul(out=scps[:, j * 16:(j + 1) * 16],
                                             lhsT=tts[:], rhs=wblk[:],
                                             start=True, stop=True)
                        # batched score tail over gn tiles
                        sc3 = scps[:, 0:gn * 16].rearrange("p (g k) -> p g k", k=16)
                        av = a_sb[:, t0 + gt0:t0 + gt0 + gn]
                        u = scbpool.tile([P, GB * 8], F32, tag="u")
                        u3 = u[:, 0:gn * 8].rearrange("p (g k) -> p g k", k=8)
                        nc.vector.tensor_tensor(
                            out=u3, in0=sc3[:, :, 0:8],
                            in1=av[:, :, None].to_broadcast((P, gn, 8)),
                            op=AX.mult)
                        sc = scbpool.tile([P, GB * 8], F32, tag="sc")
                        sc3b = sc[:, 0:gn * 8].rearrange("p (g k) -> p g k", k=8)
                        nc.vector.tensor_tensor(out=sc3b, in0=u3,
                                                in1=sc3[:, :, 8:16], op=AX.add)
                        nc.vector.tensor_scalar(out=sc[:, 0:gn * 8],
                                                in0=sc[:, 0:gn * 8],
                                                scalar1=5.0, scalar2=-5.0,
                                                op0=AX.min, op1=AX.max)
                        exps = scbpool.tile([P, GB * 8], BF16, tag="exps")
                        nc.scalar.activation(out=exps[:, 0:gn * 8],
                                             in_=sc[:, 0:gn * 8], func=AF.Exp)
                        for j in range(gn):
                            ti = gt0 + j
                            kvg = kvgs[j]
                            msgz = wpool.tile([P, 136], BF16, tag="msgz")
                            nc.vector.tensor_tensor(
                                out=msgz[:].rearrange("p (h d) -> p h d", d=17),
                                in0=kvg[:, 128:ROW].rearrange(
                                    "p (h d) -> p h d", d=17),
                                in1=exps[:, j * 8:(j + 1) * 8][:, :, None]
                                    .to_broadcast((P, 8, 17)),
                                op=AX.mult)
                            nc.tensor.matmul(out=acc[:],
                                             lhsT=ohb[:, ti * P:(ti + 1) * P],
                                             rhs=msgz[:],
                                             start=(ti == 0),
                                             stop=(ti == nt - 1))
                    # readout: interleaved acc = per head [wV'(16) | Z(1)];
                    # h = (wV' + bv*Z) / (Z + eps)
                    a3 = acc[:].rearrange("p (h d) -> p h d", d=17)
                    zp = wpool.tile([P, 8], F32, tag="zp")
                    nc.vector.tensor_scalar(out=zp[:], in0=a3[:, :, 16],
                                            scalar1=1e-6, scalar2=None, op0=AX.add)
                    rt = wpool.tile([P, 8], F32, tag="rt")
                    nc.vector.reciprocal(out=rt[:], in_=zp[:])
                    bz = wpool.tile([P, P], F32, tag="bz")
                    nc.vector.tensor_tensor(
                        out=bz[:].rearrange("p (h d) -> p h d", d=16),
                        in0=bvb[:].rearrange("p (h d) -> p h d", d=16),
                        in1=a3[:, :, 16:17].to_broadcast((P, 8, 16)),
                        op=AX.mult)
                    wv = wpool.tile([P, P], F32, tag="wv")
                    nc.vector.tensor_tensor(
                        out=wv[:].rearrange("p (h d) -> p h d", d=16),
                        in0=a3[:, :, 0:16],
                        in1=bz[:].rearrange("p (h d) -> p h d", d=16),
                        op=AX.add)
                    ho = wpool.tile([P, P], F32, tag="ho")
                    nc.vector.tensor_tensor(
                        out=ho[:].rearrange("p (h d) -> p h d", d=16),
                        in0=wv[:].rearrange("p (h d) -> p h d", d=16),
                        in1=rt[:][:, :, None].to_broadcast((P, 8, 16)),
                        op=AX.mult)
                    nc.sync.dma_start(out=HOUT[blk * P:(blk + 1) * P, :], in_=ho[:])
    return nc


def host_prep(x, eidx, eattr, cfg: Cfg):
    """Index-only edge prep + dtype/layout prep of inputs. Returns in_maps."""
    n, _ = x.shape
    src = np.asarray(eidx[0], dtype=np.int64)
    dst = np.asarray(eidx[1], dtype=np.int64)
    a = np.asarray(eattr, dtype=np.float32).reshape(-1)

    order = np.argsort(dst, kind="stable")
    src_s, dst_s, a_s = src[order], dst[order], a[order]
    core_of = np.minimum(dst_s // cfg.npc, cfg.n_cores - 1)

    per_core = []
    cntmax = np.zeros(cfg.nb, dtype=np.int64)
    for c in range(cfg.n_cores):
        m = core_of == c
        s, d, av = src_s[m], dst_s[m] - c * cfg.npc, a_s[m]
        blk = d >> 7
        cnt = np.bincount(blk, minlength=cfg.nb)
        cntmax = np.maximum(cntmax, cnt)
        per_core.append((s, d, av, blk, cnt))
    # variable tiles per block: max over cores per block index
    ntb = np.maximum(np.ceil(cntmax / P).astype(np.int64), 1)
    cfg.ntb = [int(v) for v in ntb]
    base = np.zeros(cfg.nb, dtype=np.int64)
    base[1:] = np.cumsum(ntb)[:-1]
    cfg.base = [int(v) for v in base]
    cfg.tt = int(ntb.sum())

    xt_pad = np.zeros((cfg.npad, x.shape[1]), dtype=np.float32)
    xt_pad[:n] = x
    xt_bf = np.ascontiguousarray(xt_pad.T).astype(ml_dtypes.bfloat16)

    in_maps = []
    for c in range(cfg.n_cores):
        s, d, av, blk, cnt = per_core[c]
        starts = np.zeros(cfg.nb, dtype=np.int64)
        starts[1:] = np.cumsum(cnt)[:-1]
        pos = np.arange(len(s)) - starts[blk]
        slot = base[blk] * P + pos

        tot = cfg.tt * P
        SRCa = np.zeros(tot, dtype=np.int32)
        Aa = np.zeros(tot, dtype=np.float32)
        IBa = np.full(tot, 200, dtype=np.int32)
        SRCa[slot] = s
        Aa[slot] = av
        IBa[slot] = (d - blk * P).astype(np.int32)

        # one-hots: A[T, e, c] = (ib[T,e] == c)
        ibm = IBa.reshape(cfg.tt, P)
        A = (ibm[:, :, None] == np.arange(P)[None, None, :])
        OHa = np.ascontiguousarray(
            A.transpose(1, 0, 2)).reshape(P, cfg.tt * P)      # [e, T*128+c]
        OH2a = np.ascontiguousarray(
            A.transpose(2, 0, 1)).reshape(P, cfg.tt * P)      # [c, T*128+e]

        def tmat(arr):
            return np.ascontiguousarray(arr.reshape(cfg.tt, P).T)

        in_maps.append({
            "xt": xt_bf,
            "xtq": np.ascontiguousarray(
                xt_bf[:, c * cfg.npc:c * cfg.npc + cfg.qrows]),
            "srct": tmat(SRCa),
            "at": tmat(Aa),
            "oht": OHa.astype(ml_dtypes.float8_e4m3),
            "oh2t": OH2a.astype(ml_dtypes.float8_e4m3),
        })
    return in_maps


def _weight_maps(Wq, bq, Wk, bk, We, be, Wv, bv):
    bf = ml_dtypes.bfloat16
    wkv = np.concatenate([Wk, Wv], axis=1).astype(np.float32)
    # Wblk = block-diag(We/4 | be/4): [128, 16]
    wblk = np.zeros((128, 16), dtype=np.float32)
    We = We.reshape(-1)
    be = be.reshape(-1)
    for h in range(8):
        wblk[16 * h:16 * h + 16, h] = We[16 * h:16 * h + 16] * 0.25
        wblk[16 * h:16 * h + 16, 8 + h] = be[16 * h:16 * h + 16] * 0.25
    return {
        "wkv": np.ascontiguousarray(wkv).astype(bf),
        "wq": np.ascontiguousarray(Wq.astype(np.float32)).astype(bf),
        "bqr": bq.astype(np.float32).reshape(1, -1).astype(bf),
        "onesr": np.ones((1, 128), dtype=np.float32).astype(bf),
        "bkb": np.ascontiguousarray(
            np.broadcast_to(bk.astype(np.float32), (128, 128))),
        "bvb": np.ascontiguousarray(
            np.broadcast_to(bv.astype(np.float32), (128, 128))),
        "idn": np.eye(128, dtype=np.float32).astype(bf),
        "wblk": wblk.astype(bf),
    }


def kernel(**inputs):
    global LAST_RESULTS
    _install_wait_fix()
    x = np.asarray(inputs["x"], dtype=np.float32)
    cfg = Cfg(n_nodes=x.shape[0])
    in_maps = host_prep(x, inputs["expander_edge_index"],
                        inputs["expander_edge_attr"], cfg)
    wm = _weight_maps(np.asarray(inputs["Wq"]), np.asarray(inputs["bq"]),
                      np.asarray(inputs["Wk"]), np.asarray(inputs["bk"]),
                      np.asarray(inputs["We"]), np.asarray(inputs["be"]),
                      np.asarray(inputs["Wv"]), np.asarray(inputs["bv"]))
    for im in in_maps:
        im.update(wm)

    nc = build_nc(cfg)
    trace = os.environ.get("KERNEL_TRACE", "0") == "1"
    res = run_bass_kernel_spmd(nc, in_maps, list(range(cfg.n_cores)), trace=trace)
    LAST_RESULTS = res

    out = np.empty((x.shape[0], x.shape[1]), dtype=np.float32)
    for c in range(cfg.n_cores):
        out[c * cfg.npc:(c + 1) * cfg.npc] = res.results[c]["hout"][:cfg.npc]
    return out


# revision 13
# speedup vs baseline: 1.3317x; 1.3317x over previous
"""Trainium2 Bass kernel for Exphormer sparse attention (GNN message passing).

Strategy (v3):
  - Nodes range-partitioned across 8 cores by dst; edges routed to the core
    owning their dst and grouped into 128-node dst blocks, so per-core
    segment sums are complete (no collective).
  - DRAM node table KVT[n, 264] bf16 per core: row = [K(128) | VI(136)],
    where VI interleaves V with ones (per head: 16 V dims then a 1.0), so a
    single per-edge multiply produces the fused [msg | Z] accumulation
    operand.  Biases ride a one-time broadcast prefill of KVT; the per-tile
    table write uses a CCE-add DMA, so no bias matmuls.
  - Per 128-edge tile: KV[src] rows via one indirect-DMA gather; Q[dst]
    expanded by a one-hot matmul.  One-hot matrices (both orientations) are
    precomputed on the host from the edge indices and DMA'd as fp8 -- no
    on-device one-hot builds.
  - T = K[src]*Q[dst] (DVE); transpose(T) (PE); T^T @ Wblk -> per-head
    [dw|db] into a grouped PSUM tile.  Score tail sc=clip(dw*a+db), exp
    batched over 6 tiles (4 instructions per group).
  - msg|Z = VI[src]*exp(sc) in one DVE op; segment-sum via one-hot matmul
    accumulated in PSUM per dst block.  Tiles per block are variable
    (max over cores per block index) to cut padding.
"""

import math
import os

import numpy as np
import ml_dtypes
import orjson

import concourse.bass as bass
import concourse.tile as tile
import concourse.mybir as mybir
from concourse.bass_utils import run_bass_kernel_spmd

_WAIT_LIMIT = 1
_fix_installed = False


def _split_waits(bir_json: bytes, limit: int = _WAIT_LIMIT) -> bytes:
    d = orjson.loads(bir_json)
    ctr = 0
    changed = False
    for fn in d.get("functions", []):
        for blk in fn.get("blocks", []):
            out = []
            for inst in blk.get("instructions", []):
                si = inst.get("sync_info")
                ow = (si or {}).get("on_wait") or []
                if si is not None and len(ow) > limit and "engine" in inst:
                    keep = ow[-limit:]
                    for w in ow[:-limit]:
                        ctr += 1
                        out.append({
                            "debug": inst.get("debug", 0),
                            "engine": inst["engine"],
                            "ins": [],
                            "outs": [],
                            "name": f"antsw-{ctr}-{inst['name']}",
                            "opcode": "EventSemaphore",
                            "sync_info": {"on_update": [], "on_wait": [w]},
                        })
                    si["on_wait"] = keep
                    changed = True
                out.append(inst)
            blk["instructions"] = out
    if not changed:
        return bir_json
    return orjson.dumps(d)


def _install_wait_fix():
    global _fix_installed
    if _fix_installed:
        return
    _fix_installed = True
    import concourse.bass_utils as bu
    import concourse.bass2jax as b2j

    orig = bu.compile_bir_kernel

    def wrapped(bir_json, tmpdir, neff_name="file.neff"):
        return orig(_split_waits(bytes(bir_json)), tmpdir, neff_name=neff_name)

    bu.compile_bir_kernel = wrapped
    b2j.compile_bir_kernel = wrapped


P = 128
ROW = 264          # table row: K(128) | VI(136)
F32 = mybir.dt.float32
BF16 = mybir.dt.bfloat16
FP8 = mybir.dt.float8e4
I32 = mybir.dt.int32
AX = mybir.AluOpType
AF = mybir.ActivationFunctionType

LAST_RESULTS = None  # test harness reads exec_time_ns from here


class Cfg:
    def __init__(self, n_nodes=50000, n_cores=8):
        self.n_nodes = n_nodes
        self.n_cores = n_cores
        self.npc = n_nodes // n_cores            # nodes per core
        self.nb = math.ceil(self.npc / P)        # dst blocks per core
        self.qrows = self.nb * P                 # padded local Q rows
        need = max(n_nodes, (n_cores - 1) * self.npc + self.qrows)
        self.npad = math.ceil(need / P) * P
        self.ntb = [1] * self.nb                 # tiles per block (host fills)
        self.base = [0] * self.nb                # tile base per block
        self.tt = self.nb                        # total edge tiles per core


def build_nc(cfg: Cfg):
    kvgbufs = int(os.environ.get("KERNEL_KVGBUFS", "24"))
    GB = int(os.environ.get("KERNEL_GB", "6"))   # tiles per score group
    nc = bass.Bass("TRN2", target_bir_lowering=False, num_devices=cfg.n_cores)

    XT = nc.dram_tensor("xt", [P, cfg.npad], BF16, kind="ExternalInput").ap()
    XTQ = nc.dram_tensor("xtq", [P, cfg.qrows], BF16, kind="ExternalInput").ap()
    WKV = nc.dram_tensor("wkv", [P, 256], BF16, kind="ExternalInput").ap()
    WQ = nc.dram_tensor("wq", [P, P], BF16, kind="ExternalInput").ap()
    BQR = nc.dram_tensor("bqr", [1, P], BF16, kind="ExternalInput").ap()
    ONESR = nc.dram_tensor("onesr", [1, P], BF16, kind="ExternalInput").ap()
    BKB = nc.dram_tensor("bkb", [P, P], F32, kind="ExternalInput").ap()
    BVB = nc.dram_tensor("bvb", [P, P], F32, kind="ExternalInput").ap()
    IDN = nc.dram_tensor("idn", [P, P], BF16, kind="ExternalInput").ap()
    WBLK = nc.dram_tensor("wblk", [P, 16], BF16, kind="ExternalInput").ap()
    SRC = nc.dram_tensor("srct", [P, cfg.tt], I32, kind="ExternalInput").ap()
    AT = nc.dram_tensor("at", [P, cfg.tt], F32, kind="ExternalInput").ap()
    OH2T = nc.dram_tensor("oh2t", [P, cfg.tt * P], FP8, kind="ExternalInput").ap()
    OHT = nc.dram_tensor("oht", [P, cfg.tt * P], FP8, kind="ExternalInput").ap()
    HOUT = nc.dram_tensor("hout", [cfg.qrows, P], F32, kind="ExternalOutput").ap()

    KVT = nc.dram_tensor("kvt", [cfg.npad, ROW], BF16).ap()

    n_kv_tiles = cfg.npad // P
    n_q_tiles = cfg.qrows // P
    SLAB = 8  # node tiles per x-slab load

    with tile.TileContext(nc) as tc:
        with (
            tc.tile_pool(name="const", bufs=1) as cpool,
            tc.tile_pool(name="meta", bufs=1) as mpool,
            tc.tile_pool(name="qres", bufs=1) as qpool,
        ):
            # ---- constants (all host-prepped) ----
            ident = cpool.tile([P, P], BF16)
            nc.sync.dma_start(out=ident[:], in_=IDN[:])
            ones_row = cpool.tile([1, P], BF16)
            nc.sync.dma_start(out=ones_row[:], in_=ONESR[:])
            wkv_sb = cpool.tile([P, 256], BF16)
            nc.sync.dma_start(out=wkv_sb[:], in_=WKV[:])
            wq_sb = cpool.tile([P, P], BF16)
            nc.sync.dma_start(out=wq_sb[:], in_=WQ[:])
            bq_sb = cpool.tile([1, P], BF16)
            nc.sync.dma_start(out=bq_sb[:], in_=BQR[:])
            wblk = cpool.tile([P, 16], BF16)
            nc.sync.dma_start(out=wblk[:], in_=WBLK[:])
            bkb = cpool.tile([P, P], F32)
            nc.sync.dma_start(out=bkb[:], in_=BKB[:])
            bvb = cpool.tile([P, P], F32)
            nc.sync.dma_start(out=bvb[:], in_=BVB[:])

            # ---- resident edge metadata ----
            src_sb = mpool.tile([P, cfg.tt], I32)
            nc.sync.dma_start(out=src_sb[:], in_=SRC[:])
            a_sb = mpool.tile([P, cfg.tt], F32)
            nc.sync.dma_start(out=a_sb[:], in_=AT[:])

            # ---- local Q, SBUF-resident [128, nb*128] ----
            qres = qpool.tile([P, cfg.qrows], BF16)

            with (
                tc.tile_pool(name="bld", bufs=3) as bpool,
                tc.tile_pool(name="bldp", bufs=2, space="PSUM") as bppool,
            ):
                for j in range(n_q_tiles):
                    if j % SLAB == 0:
                        xq = bpool.tile([P, SLAB * P], BF16, tag="xq")
                        w = min(SLAB * P, cfg.qrows - j * P)
                        nc.sync.dma_start(out=xq[:, :w], in_=XTQ[:, j * P:j * P + w])
                    lhs = xq[:, (j % SLAB) * P:(j % SLAB + 1) * P]
                    qp = bppool.tile([P, P], F32, tag="qp")
                    nc.tensor.matmul(out=qp[:], lhsT=lhs, rhs=wq_sb[:],
                                     start=True, stop=False)
                    nc.tensor.matmul(out=qp[:], lhsT=ones_row[:1, :], rhs=bq_sb[:1, :],
                                     start=False, stop=True)
                    if j % 2 == 0:
                        nc.scalar.activation(out=qres[:, j * P:(j + 1) * P],
                                             in_=qp[:], func=AF.Copy)
                    else:
                        nc.vector.tensor_copy(out=qres[:, j * P:(j + 1) * P],
                                              in_=qp[:])

                for i in range(n_kv_tiles):
                    if i % SLAB == 0:
                        xs = bpool.tile([P, SLAB * P], BF16, tag="xs")
                        w = min(SLAB * P, cfg.npad - i * P)
                        nc.sync.dma_start(out=xs[:, :w], in_=XT[:, i * P:i * P + w])
                    lhs = xs[:, (i % SLAB) * P:(i % SLAB + 1) * P]
                    bp = bppool.tile([P, 256], F32, tag="bp")
                    nc.tensor.matmul(out=bp[:], lhsT=lhs, rhs=wkv_sb[:],
                                     start=True, stop=True)
                    kvsb = bpool.tile([P, ROW], BF16, tag="kvsb")
                    vi = kvsb[:, 128:ROW].rearrange("p (h d) -> p h d", d=17)
                    if i < 3:  # set the ones-slot once per pool buffer
                        nc.gpsimd.memset(vi[:, :, 16:17], 1.0)
                    # K with bias (DVE add); V without bias (bias rides Z at
                    # readout: h = (wV' + bv*Z)/(Z+eps))
                    nc.vector.tensor_tensor(out=kvsb[:, 0:128], in0=bp[:, 0:128],
                                            in1=bkb[:], op=AX.add)
                    nc.scalar.activation(
                        out=vi[:, :, 0:16],
                        in_=bp[:, 128:256].rearrange("p (h d) -> p h d", d=16),
                        func=AF.Copy)
                    nc.sync.dma_start(out=KVT[i * P:(i + 1) * P, :], in_=kvsb[:])

            # ---- edge phase ----
            with (
                tc.tile_pool(name="kvg", bufs=kvgbufs) as kvpool,
                tc.tile_pool(name="work", bufs=10) as wpool,
                tc.tile_pool(name="ohb", bufs=3) as ohpool,
                tc.tile_pool(name="scb", bufs=4) as scbpool,
                tc.tile_pool(name="qxp", bufs=2, space="PSUM") as qxpool,
                tc.tile_pool(name="ttp", bufs=2, space="PSUM") as ttpool,
                tc.tile_pool(name="scp", bufs=2, space="PSUM") as scpool,
                tc.tile_pool(name="accp", bufs=2, space="PSUM") as accpool,
            ):
                ntmax = max(cfg.ntb)
                for blk in range(cfg.nb):
                    t0 = cfg.base[blk]
                    nt = cfg.ntb[blk]
                    qwin = qres[:, blk * P:(blk + 1) * P]
                    oh2b = ohpool.tile([P, ntmax * P], FP8, tag="oh2b")
                    nc.sync.dma_start(out=oh2b[:, 0:nt * P],
                                      in_=OH2T[:, t0 * P:(t0 + nt) * P])
                    ohb = ohpool.tile([P, ntmax * P], FP8, tag="ohb")
                    nc.sync.dma_start(out=ohb[:, 0:nt * P],
                                      in_=OHT[:, t0 * P:(t0 + nt) * P])
                    acc = accpool.tile([P, 136], F32, tag="acc")
                    n_grp = math.ceil(nt / GB)
                    for g in range(n_grp):
                        gt0 = g * GB
                        gn = min(GB, nt - gt0)
                        scps = scpool.tile([P, GB * 16], F32, tag="scps")
                        kvgs = []
                        for j in range(gn):
                            ti = gt0 + j
                            T = t0 + ti
                            kvg = kvpool.tile([P, ROW], BF16, tag="kvg")
                            kvgs.append(kvg)
                            nc.gpsimd.indirect_dma_start(
                                out=kvg[:], out_offset=None, in_=KVT[:],
                                in_offset=bass.IndirectOffsetOnAxis(
                                    ap=src_sb[:, T:T + 1], axis=0))
                            qx = qxpool.tile([P, P], F32, tag="qx")
                            nc.tensor.matmul(out=qx[:],
                                             lhsT=oh2b[:, ti * P:(ti + 1) * P],
                                             rhs=qwin, start=True, stop=True)
                            ttile = wpool.tile([P, P], BF16, tag="ttile")
                            nc.vector.tensor_tensor(
                                out=ttile[:], in0=kvg[:, 0:128],
                                in1=qx[:], op=AX.mult)
                            ttp = ttpool.tile([P, P], BF16, tag="ttp")
                            nc.tensor.transpose(out=ttp[:], in_=ttile[:],
                                                identity=ident[:])
                            tts = wpool.tile([P, P], BF16, tag="tts")
                            nc.scalar.activation(out=tts[:], in_=ttp[:], func=AF.Copy)
                            nc.tensor.matmul(out=scps[:, j * 16:(j + 1) * 16],
                                             lhsT=tts[:], rhs=wblk[:],
                                             start=True, stop=True)
                        # batched score tail over gn tiles
                        sc3 = scps[:, 0:gn * 16].rearrange("p (g k) -> p g k", k=16)
                        av = a_sb[:, t0 + gt0:t0 + gt0 + gn]
                        u = scbpool.tile([P, GB * 8], F32, tag="u")
                        u3 = u[:, 0:gn * 8].rearrange("p (g k) -> p g k", k=8)
                        nc.vector.tensor_tensor(
                            out=u3, in0=sc3[:, :, 0:8],
                            in1=av[:, :, None].to_broadcast((P, gn, 8)),
                            op=AX.mult)
                        sc = scbpool.tile([P, GB * 8], F32, tag="sc")
                        sc3b = sc[:, 0:gn * 8].rearrange("p (g k) -> p g k", k=8)
                        nc.vector.tensor_tensor(out=sc3b, in0=u3,
                                                in1=sc3[:, :, 8:16], op=AX.add)
                        nc.vector.tensor_scalar(out=sc[:, 0:gn * 8],
                                                in0=sc[:, 0:gn * 8],
                                                scalar1=5.0, scalar2=-5.0,
                                                op0=AX.min, op1=AX.max)
                        exps = scbpool.tile([P, GB * 8], BF16, tag="exps")
                        nc.scalar.activation(out=exps[:, 0:gn * 8],
                                             in_=sc[:, 0:gn * 8], func=AF.Exp)
                        for j in range(gn):
                            ti = gt0 + j
                            kvg = kvgs[j]
                            msgz = wpool.tile([P, 136], BF16, tag="msgz")
                            nc.vector.tensor_tensor(
                                out=msgz[:].rearrange("p (h d) -> p h d", d=17),
                                in0=kvg[:, 128:ROW].rearrange(
                                    "p (h d) -> p h d", d=17),
                                in1=exps[:, j * 8:(j + 1) * 8][:, :, None]
                                    .to_broadcast((P, 8, 17)),
                                op=AX.mult)
                            nc.tensor.matmul(out=acc[:],
                                             lhsT=ohb[:, ti * P:(ti + 1) * P],
                                             rhs=msgz[:],
                                             start=(ti == 0),
                                             stop=(ti == nt - 1))
                    # readout: interleaved acc = per head [wV'(16) | Z(1)];
                    # h = (wV' + bv*Z) / (Z + eps)
                    a3 = acc[:].rearrange("p (h d) -> p h d", d=17)
                    zp = wpool.tile([P, 8], F32, tag="zp")
                    nc.vector.tensor_scalar(out=zp[:], in0=a3[:, :, 16],
                                            scalar1=1e-6, scalar2=None, op0=AX.add)
                    rt = wpool.tile([P, 8], F32, tag="rt")
                    nc.vector.reciprocal(out=rt[:], in_=zp[:])
                    bz = wpool.tile([P, P], F32, tag="bz")
                    nc.vector.tensor_tensor(
                        out=bz[:].rearrange("p (h d) -> p h d", d=16),
                        in0=bvb[:].rearrange("p (h d) -> p h d", d=16),
                        in1=a3[:, :, 16:17].to_broadcast((P, 8, 16)),
                        op=AX.mult)
                    wv = wpool.tile([P, P], F32, tag="wv")
                    nc.vector.tensor_tensor(
                        out=wv[:].rearrange("p (h d) -> p h d", d=16),
                        in0=a3[:, :, 0:16],
                        in1=bz[:].rearrange("p (h d) -> p h d", d=16),
                        op=AX.add)
                    ho = wpool.tile([P, P], F32, tag="ho")
                    nc.vector.tensor_tensor(
                        out=ho[:].rearrange("p (h d) -> p h d", d=16),
                        in0=wv[:].rearrange("p (h d) -> p h d", d=16),
                        in1=rt[:][:, :, None].to_broadcast((P, 8, 16)),
                        op=AX.mult)
                    nc.sync.dma_start(out=HOUT[blk * P:(blk + 1) * P, :], in_=ho[:])
    return nc


def host_prep(x, eidx, eattr, cfg: Cfg):
    """Index-only edge prep + dtype/layout prep of inputs. Returns in_maps."""
    n, _ = x.shape
    src = np.asarray(eidx[0], dtype=np.int64)
    dst = np.asarray(eidx[1], dtype=np.int64)
    a = np.asarray(eattr, dtype=np.float32).reshape(-1)

    order = np.argsort(dst, kind="stable")
    src_s, dst_s, a_s = src[order], dst[order], a[order]
    core_of = np.minimum(dst_s // cfg.npc, cfg.n_cores - 1)

    per_core = []
    cnts = np.zeros((cfg.n_cores, cfg.nb), dtype=np.int64)
    for c in range(cfg.n_cores):
        m = core_of == c
        s, d, av = src_s[m], dst_s[m] - c * cfg.npc, a_s[m]
        blk = d >> 7
        cnt = np.bincount(blk, minlength=cfg.nb)
        cnts[c] = cnt
        per_core.append((s, d, av, blk, cnt))
    # slot i of every core holds its i-th busiest block (sorted descending);
    # per-slot tile count = max over cores of the slotted block's count
    perms = [np.argsort(-cnts[c], kind="stable") for c in range(cfg.n_cores)]
    slotmax = np.max(np.stack([cnts[c][perms[c]] for c in range(cfg.n_cores)]),
                     axis=0)
    ntb = np.maximum(np.ceil(slotmax / P).astype(np.int64), 1)
    cfg.ntb = [int(v) for v in ntb]
    base = np.zeros(cfg.nb, dtype=np.int64)
    base[1:] = np.cumsum(ntb)[:-1]
    cfg.base = [int(v) for v in base]
    cfg.tt = int(ntb.sum())
    cfg.perms = perms

    xt_pad = np.zeros((cfg.npad, x.shape[1]), dtype=np.float32)
    xt_pad[:n] = x
    xt_bf = np.ascontiguousarray(xt_pad.T).astype(ml_dtypes.bfloat16)

    in_maps = []
    for c in range(cfg.n_cores):
        s, d, av, blk, cnt = per_core[c]
        slot_of_block = np.empty(cfg.nb, dtype=np.int64)
        slot_of_block[perms[c]] = np.arange(cfg.nb)
        starts = np.zeros(cfg.nb, dtype=np.int64)
        starts[1:] = np.cumsum(cnt)[:-1]
        pos = np.arange(len(s)) - starts[blk]
        slot = base[slot_of_block[blk]] * P + pos

        tot = cfg.tt * P
        SRCa = np.zeros(tot, dtype=np.int32)
        Aa = np.zeros(tot, dtype=np.float32)
        IBa = np.full(tot, 200, dtype=np.int32)
        SRCa[slot] = s
        Aa[slot] = av
        IBa[slot] = (d - blk * P).astype(np.int32)

        # one-hots: A[T, e, c] = (ib[T,e] == c)
        ibm = IBa.reshape(cfg.tt, P)
        A = (ibm[:, :, None] == np.arange(P)[None, None, :])
        OHa = np.ascontiguousarray(
            A.transpose(1, 0, 2)).reshape(P, cfg.tt * P)      # [e, T*128+c]
        OH2a = np.ascontiguousarray(
            A.transpose(2, 0, 1)).reshape(P, cfg.tt * P)      # [c, T*128+e]

        def tmat(arr):
            return np.ascontiguousarray(arr.reshape(cfg.tt, P).T)

        in_maps.append({
            "xt": xt_bf,
            "xtq": np.ascontiguousarray(
                np.concatenate(
                    [xt_bf[:, c * cfg.npc + b * P:c * cfg.npc + b * P + P]
                     if (c * cfg.npc + b * P) < xt_bf.shape[1] else
                     np.zeros((P, P), dtype=xt_bf.dtype)
                     for b in perms[c]], axis=1)),
            "srct": tmat(SRCa),
            "at": tmat(Aa),
            "oht": OHa.astype(ml_dtypes.float8_e4m3),
            "oh2t": OH2a.astype(ml_dtypes.float8_e4m3),
        })
    return in_maps


def _weight_maps(Wq, bq, Wk, bk, We, be, Wv, bv):
    bf = ml_dtypes.bfloat16
    wkv = np.concatenate([Wk, Wv], axis=1).astype(np.float32)
    # Wblk = block-diag(We/4 | be/4): [128, 16]
    wblk = np.zeros((128, 16), dtype=np.float32)
    We = We.reshape(-1)
    be = be.reshape(-1)
    for h in range(8):
        wblk[16 * h:16 * h + 16, h] = We[16 * h:16 * h + 16] * 0.25
        wblk[16 * h:16 * h + 16, 8 + h] = be[16 * h:16 * h + 16] * 0.25
    return {
        "wkv": np.ascontiguousarray(wkv).astype(bf),
        "wq": np.ascontiguousarray(Wq.astype(np.float32)).astype(bf),
        "bqr": bq.astype(np.float32).reshape(1, -1).astype(bf),
        "onesr": np.ones((1, 128), dtype=np.float32).astype(bf),
        "bkb": np.ascontiguousarray(
            np.broadcast_to(bk.astype(np.float32), (128, 128))),
        "bvb": np.ascontiguousarray(
            np.broadcast_to(bv.astype(np.float32), (128, 128))),
        "idn": np.eye(128, dtype=np.float32).astype(bf),
        "wblk": wblk.astype(bf),
    }


def kernel(**inputs):
    global LAST_RESULTS
    _install_wait_fix()
    x = np.asarray(inputs["x"], dtype=np.float32)
    cfg = Cfg(n_nodes=x.shape[0])
    in_maps = host_prep(x, inputs["expander_edge_index"],
                        inputs["expander_edge_attr"], cfg)
    wm = _weight_maps(np.asarray(inputs["Wq"]), np.asarray(inputs["bq"]),
                      np.asarray(inputs["Wk"]), np.asarray(inputs["bk"]),
                      np.asarray(inputs["We"]), np.asarray(inputs["be"]),
                      np.asarray(inputs["Wv"]), np.asarray(inputs["bv"]))
    for im in in_maps:
        im.update(wm)

    nc = build_nc(cfg)
    trace = os.environ.get("KERNEL_TRACE", "0") == "1"
    res = run_bass_kernel_spmd(nc, in_maps, list(range(cfg.n_cores)), trace=trace)
    LAST_RESULTS = res

    out = np.empty((x.shape[0], x.shape[1]), dtype=np.float32)
    for c in range(cfg.n_cores):
        h = res.results[c]["hout"]
        for i, b in enumerate(cfg.perms[c]):
            lo = c * cfg.npc + b * P
            hi = min(lo + P, (c + 1) * cfg.npc, x.shape[0])
            if hi > lo:
                out[lo:hi] = h[i * P:i * P + (hi - lo)]
    return out
